# revision 27
# baseline (speedup 1.0000x reference)
"""AttentionBlock (GroupNorm + 8-head self-attention + proj + residual) on 8 TRN2 cores.

Sharding: pure data-parallel over batch. B=16 -> 2 images per core, no collectives.

Per-core pipeline (matmuls bf16, fp32 PSUM):
  - Weights transposed on the TENSOR engine (fp32 PE transpose via identity into
    PSUM, ACT evacuates+casts to one bf16 [128, ci, o] SBUF tile) -- no slow
    DMA-xbar transposes, and the transposes warm up the PE clock.
  - x streamed in bf16 (host-cast side input) for groupnorm stats + normalize;
    f32 x re-read only for the residual add.
  - GroupNorm stats: free-axis reduce + square-accumulate fused on DVE
    (tensor_tensor_reduce), 16-channel group sums via XOR stream_shuffle
    butterfly, rsqrt via bit-hack + Newton (all DVE, ACT keeps only exp).
  - QKV: q,k as [o, n]; v pre-transposed as [n, o_v] with a ones column for
    softmax row sums. PSUM evacuation split across DVE and ACT.
  - S^T = k^T q per head pair with tile_position row packing (concurrent 64-row
    streams); exp straight out of PSUM: most tiles on ACT (Exp activation),
    a few per pair on DVE via a Schraudolph int16 bit-trick (|S|/8 < ~80 safe).
  - A@V accumulates h^T (+sums row 64); spilled bf16 to SBUF promptly.
    Row-sum rows are gathered by SBUF->SBUF DMA into per-pair stacks,
    reciprocal via DVE reciprocal_approx_fast, broadcast along partitions by
    stride-0 SBUF DMA, one DVE multiply normalizes.
  - proj + residual (+bpe = b_proj + w_proj @ b_v folded on host).
"""

import numpy as np

import concourse.bass as bass
import concourse.tile as tile
from concourse import mybir, masks
from concourse.bass_utils import run_bass_kernel_spmd

F32 = mybir.dt.float32
I8 = mybir.dt.int8
I16 = mybir.dt.int16
I32 = mybir.dt.int32
BF16 = mybir.dt.bfloat16
FP8 = mybir.dt.float8e4
AX = mybir.AxisListType
ALU = mybir.AluOpType
ACTF = mybir.ActivationFunctionType
DR = mybir.MatmulPerfMode.DoubleRow

B_LOC = 2      # batch elements per core
C = 512
N = 1024       # H*W
NH = 8
HD = 64
G = 32         # groups
EPS = 1e-5
NCORES = 8

# P~ = exp(S/8 - ESHIFT): keeps P~ <= ~e^3.5 < 448 (fp8e4m3 max; overflow
# would produce NaN since e4m3fn has no inf). The shift cancels in the
# softmax normalization.
ESHIFT = 3.0
# Schraudolph exp -> fp8e4m3 bits: round((s/8 - ESHIFT)*(8/ln2) + (56 - 0.46)),
# written via uint8 so negative args saturate to 0 (= fp8 +0).
SCH_MUL = float(8.0 / np.log(2.0) / 8.0)
SCH_BIAS = float(7.0 * 8.0 - 0.46 - ESHIFT * 8.0 / np.log(2.0))
# (hh, mj) S^T tiles computed on DVE instead of ACT, per pair (load balance);
# heavier DVE share on the last pairs of batch 1 to shorten the exp tail.
DVE_EXP = {(1, 7)}
DVE_EXP_P2 = {(0, 6), (1, 6), (0, 7), (1, 7)}
DVE_EXP_P3 = {(0, 4), (1, 4), (0, 5), (1, 5), (0, 6), (1, 6), (0, 7), (1, 7)}


def _ap(t, offset_elems, pattern):
    a = t[:] if hasattr(t, "shape") else t
    return bass.AP(tensor=a.tensor, offset=a.offset + offset_elems, ap=pattern)


def _t(pool, shape, dt, tag, bufs=None):
    return pool.tile(shape, dt, tag=tag, name=tag, bufs=bufs)


def build_bass(split=True):
    nc = bass.Bass()
    xb_d = nc.declare_dram_parameter("xb", [B_LOC, C, N], BF16, isOutput=False)
    x_d = nc.declare_dram_parameter("x", [B_LOC, C, N], F32, isOutput=False)
    wq_d = nc.declare_dram_parameter("wqkv", [3 * C, C], F32, isOutput=False)
    bqk_d = nc.declare_dram_parameter("bqk", [2 * C], F32, isOutput=False)
    gam_d = nc.declare_dram_parameter("gam", [C], F32, isOutput=False)
    bet_d = nc.declare_dram_parameter("bet", [C], F32, isOutput=False)
    wp_d = nc.declare_dram_parameter("wproj", [C, C], F32, isOutput=False)
    bpe_d = nc.declare_dram_parameter("bpe", [C], F32, isOutput=False)
    out_d = nc.declare_dram_parameter("out", [B_LOC, C, N], F32, isOutput=True)

    with tile.TileContext(nc) as tc:
        _build_tile(tc, xb_d, x_d, wq_d, bqk_d, gam_d, bet_d, wp_d, bpe_d, out_d)
    if split:
        _split_multi_waits(nc)
    return nc


def _build_tile(tc, xb_d, x_d, wq_d, bqk_d, gam_d, bet_d, wp_d, bpe_d, out_d):
    nc = tc.nc
    from contextlib import ExitStack
    ctx = ExitStack()
    with ctx:
        wpool = ctx.enter_context(tc.tile_pool(name="wpool", bufs=1))
        xt_p = ctx.enter_context(tc.tile_pool(name="xt", bufs=2))
        nt_p = ctx.enter_context(tc.tile_pool(name="nt", bufs=2))
        qk_p = ctx.enter_context(tc.tile_pool(name="qk", bufs=1))
        v_p = ctx.enter_context(tc.tile_pool(name="vx", bufs=2))
        pt_p = ctx.enter_context(tc.tile_pool(name="pt", bufs=3))
        h_p = ctx.enter_context(tc.tile_pool(name="hs", bufs=2))
        hu_p = ctx.enter_context(tc.tile_pool(name="hu", bufs=1))
        st_p = ctx.enter_context(tc.tile_pool(name="stats", bufs=4))
        sk_p = ctx.enter_context(tc.tile_pool(name="sk", bufs=2))
        xr_p = ctx.enter_context(tc.tile_pool(name="xr", bufs=2))
        wst_p = ctx.enter_context(tc.tile_pool(name="wstage", bufs=2))
        dram_p = ctx.enter_context(tc.tile_pool(name="drp", bufs=1, space="DRAM"))
        pa_ps = ctx.enter_context(tc.tile_pool(name="pa", bufs=2, space="PSUM"))
        st_ps = ctx.enter_context(tc.tile_pool(name="stp", bufs=2, space="PSUM"))
        av_ps = ctx.enter_context(tc.tile_pool(name="avp", bufs=2, space="PSUM"))

        # ---- constants ----
        bqk_sb = _t(wpool, [128, 8], F32, "bqk")
        nc.gpsimd.dma_start(out=bqk_sb, in_=_ap(bqk_d, 0, [[1, 128], [128, 8]]))
        gam_sb = _t(wpool, [128, 4], F32, "gam")
        nc.gpsimd.dma_start(out=gam_sb, in_=_ap(gam_d, 0, [[1, 128], [128, 4]]))
        bet_sb = _t(wpool, [128, 4], F32, "bet")
        nc.gpsimd.dma_start(out=bet_sb, in_=_ap(bet_d, 0, [[1, 128], [128, 4]]))
        bpe_sb = _t(wpool, [128, 4], F32, "bpe")
        nc.gpsimd.dma_start(out=bpe_sb, in_=_ap(bpe_d, 0, [[1, 128], [128, 4]]))
        # identity via DMA diagonal scatter (affine_select is unsupported here)
        eshift_sb = _t(wpool, [128, 1], F32, "eshift")
        nc.vector.memset(eshift_sb[:], -ESHIFT)
        ident = _t(wpool, [128, 128], F32, "ident")
        id_dram = _t(dram_p, [128, 128], F32, "idd", bufs=1)
        nc.vector.memset(ident[:], 0.0)
        ones_row = _t(wpool, [1, 128], F32, "ones_row")
        nc.vector.memset(ones_row[:], 1.0)
        nc.gpsimd.dma_start(out=id_dram[:, :], in_=ident[:])
        nc.gpsimd.dma_start(out=_ap(id_dram, 0, [[129, 128]]), in_=ones_row[:])
        nc.gpsimd.dma_start(out=ident[:], in_=id_dram[:, :])

        # ---- x loads (bf16), spread across queues ----
        def emit_xload(b, engs):
            xt = []
            for i in range(4):
                t = _t(xt_p, [128, 1024], BF16, f"x{i}")
                engs[i % len(engs)].dma_start(out=t, in_=xb_d[b, i * 128:(i + 1) * 128, :])
                xt.append(t)
            return xt

        # ---- weights: DMA f32 slab -> 4 PE transposes -> ACT evac+cast ----
        # wT[c_in_chunk, ci, o] with o: 0:512 q | 512:1024 k | 1024:1536 v (bf16)
        # wTp[c_in_chunk, ci, o_proj] fp8 for the DoubleRow proj matmuls
        wT = _t(wpool, [128, 4, 1536], BF16, "wT")
        wTp = _t(wpool, [128, 4, 512], FP8, "wTp")

        def emit_wslab_dma(oi, eng):
            t = _t(wst_p, [128, 512], F32, "wst")
            if oi < 12:
                eng.dma_start(out=t, in_=wq_d[oi * 128:(oi + 1) * 128, :])
            else:
                eng.dma_start(out=t, in_=wp_d[(oi - 12) * 128:(oi - 11) * 128, :])
            return t

        def emit_wslab_transpose(oi, t):
            ps = _t(pa_ps, [128, 512], F32, "pa")
            for ci in range(4):
                nc.tensor.transpose(ps[:, ci * 128:(ci + 1) * 128],
                                    t[:, ci * 128:(ci + 1) * 128], ident[:])
            if oi < 12:
                dst = wT[:, :, oi * 128:(oi + 1) * 128]
            else:
                dst = wTp[:, :, (oi - 12) * 128:(oi - 11) * 128]
            nc.scalar.copy(out=dst, in_=ps[:].rearrange("p (c o) -> p c o", c=4))

        # ---- GroupNorm stats -> per-channel (s0, s1) affine scalars ----
        # All 4 channel-chunks batched: one [128, 2, 4] sums tile, one
        # butterfly, one [128, 4] rsqrt Newton chain.
        def emit_stats(b, xt):
            c = _t(st_p, [128, 2, 4], F32, f"cs{b}")
            for i in range(4):
                nc.vector.reduce_sum(out=c[:, 0, i:i + 1], in_=xt[i][:], axis=AX.X)
                scr = _t(st_p, [128, 1024], BF16, "sqscr", bufs=1)
                nc.scalar.activation(out=scr[:], in_=xt[i][:], func=ACTF.Square,
                                     accum_out=c[:, 1, i:i + 1])
            sh = _t(st_p, [128, 2, 4], F32, f"sh{b}")
            for s in (8, 4, 2, 1):
                nc.vector.stream_shuffle(out=sh, in_=c[:],
                                         mask=[j ^ s for j in range(32)])
                nc.vector.tensor_add(out=c, in0=c[:], in1=sh[:])
            mmt = _t(st_p, [128, 2, 4], F32, f"mmt{b}")
            nc.vector.tensor_scalar_mul(out=mmt, in0=c[:], scalar1=1.0 / (16 * N))
            u = _t(st_p, [128, 4], F32, f"u{b}")
            nc.vector.tensor_mul(out=u, in0=mmt[:, 0, :], in1=mmt[:, 0, :])
            nc.vector.tensor_sub(out=u, in0=mmt[:, 1, :], in1=u[:])
            nc.vector.tensor_scalar_add(out=u, in0=u[:], scalar1=EPS)
            ri = _t(st_p, [128, 4], I32, f"ri{b}")
            nc.vector.tensor_scalar(out=ri, in0=u[:].bitcast(I32), scalar1=1,
                                    scalar2=None, op0=ALU.logical_shift_right)
            nc.vector.tensor_scalar(out=ri, in0=ri[:], scalar1=-1,
                                    scalar2=0x5F3759DF, op0=ALU.mult, op1=ALU.add)
            r = ri[:].bitcast(F32)
            h = _t(st_p, [128, 4], F32, f"h{b}")
            nc.vector.tensor_scalar_mul(out=h, in0=u[:], scalar1=0.5)
            t2 = _t(st_p, [128, 4], F32, f"t2{b}")
            for _ in range(2):
                nc.vector.tensor_mul(out=t2, in0=r, in1=r)
                nc.vector.tensor_mul(out=t2, in0=h[:], in1=t2[:])
                nc.vector.tensor_scalar(out=t2, in0=t2[:], scalar1=-1.0,
                                        scalar2=1.5, op0=ALU.mult, op1=ALU.add)
                nc.vector.tensor_mul(out=r, in0=r, in1=t2[:])
            s0 = _t(st_p, [128, 4], F32, f"s0{b}")
            nc.vector.tensor_mul(out=s0, in0=gam_sb[:], in1=r)
            t1 = _t(st_p, [128, 4], F32, f"t1{b}")
            nc.vector.tensor_mul(out=t1, in0=mmt[:, 0, :], in1=s0[:])
            s1 = _t(st_p, [128, 4], F32, f"s1{b}")
            nc.vector.tensor_sub(out=s1, in0=bet_sb[:], in1=t1[:])
            return [(s0[:, i:i + 1], s1[:, i:i + 1]) for i in range(4)]

        def emit_norm_i(b, xt, s01, i):
            t = _t(nt_p, [128, 1024], BF16, f"n{i}")
            nc.vector.tensor_scalar(
                out=t[:], in0=xt[i][:], scalar1=s01[i][0], scalar2=s01[i][1],
                op0=ALU.mult, op1=ALU.add)
            return t

        def emit_qk_oi(nt, qk, oi, evac_scalar=False):
            t = _t(qk_p, [128, 1024], BF16, f"qk{oi}")
            for nj in range(2):
                pk = _t(pa_ps, [128, 512], F32, "pa")
                for ki in range(4):
                    nc.tensor.matmul(
                        pk[:], wT[:, ki, oi * 128:(oi + 1) * 128],
                        nt[ki][:, nj * 512:(nj + 1) * 512],
                        start=(ki == 0), stop=(ki == 3))
                if evac_scalar:
                    nc.scalar.add(out=t[:, nj * 512:(nj + 1) * 512], in_=pk[:],
                                  add=bqk_sb[:, oi:oi + 1])
                else:
                    nc.vector.tensor_scalar_add(
                        out=t[:, nj * 512:(nj + 1) * 512], in0=pk[:],
                        scalar1=bqk_sb[:, oi:oi + 1])
            qk[oi] = t

        def emit_vinit(b):
            """vx layout [128(kpos), kj 8, head 8, 80] fp8; col 64 = ones for
            softmax row sums, cols 65:80 pad for DoubleRow 16B stride."""
            t = _t(v_p, [128, 8, NH, 80], FP8, "vxb")
            nc.vector.memset(t[:, :, :, HD:HD + 1], 1.0)
            return t

        def emit_v_ni(nt, vxb, ni):
            pv = _t(pa_ps, [128, 512], F32, "pa")
            for ki in range(4):
                nc.tensor.matmul(
                    pv[:], nt[ki][:, ni * 128:(ni + 1) * 128],
                    wT[:, ki, 1024:1536],
                    start=(ki == 0), stop=(ki == 3))
            nc.vector.tensor_copy(
                out=vxb[:, ni, :, 0:HD],
                in_=pv[:].rearrange("p (h d) -> p h d", h=NH))

        def emit_spair(qk, hp, dve_set):
            """S^T + exp for head pair hp -> P~ fp8 tiles [128, mj 8, 1024]."""
            pb = [_t(pt_p, [128, 8, 1024], FP8, f"ptb{hh}") for hh in range(2)]
            for mj in range(8):
                stt = [None, None]
                for hh in range(2):
                    base = 64 * hh
                    stt[hh] = _t(st_ps, [128, 1024], F32, "st")
                    for ni in range(2):
                        nc.tensor.matmul(
                            stt[hh][:, ni * 512:(ni + 1) * 512],
                            qk[4 + hp][base:base + 64, mj * 128:(mj + 1) * 128],
                            qk[hp][base:base + 64, ni * 512:(ni + 1) * 512],
                            start=True, stop=True, tile_position=(base, 0))
                for hh in range(2):
                    p = pb[hh][:, mj, :]
                    if (hh, mj) in dve_set:
                        nc.vector.tensor_scalar(
                            out=p.bitcast(mybir.dt.uint8), in0=stt[hh][:],
                            scalar1=SCH_MUL, scalar2=SCH_BIAS,
                            op0=ALU.mult, op1=ALU.add)
                    else:
                        nc.scalar.activation(out=p, in_=stt[hh][:], func=ACTF.Exp,
                                             scale=float(HD) ** -0.5,
                                             bias=eshift_sb[:])
            return pb

        def emit_avpair(vxb, pb, hp, stk, hu_all, dq):
            """A@V (fp8 DoubleRow) -> spill h^T (unnormalized) + sums."""
            for ni in range(2):
                for hh in range(2):
                    pav = _t(av_ps, [HD + 1, 512], F32, "av")
                    for kjp in range(0, 8, 2):
                        nc.tensor.matmul(
                            pav[:], vxb[:, kjp:kjp + 2, 2 * hp + hh, 0:HD + 1],
                            pb[hh][:, kjp:kjp + 2, ni * 512:(ni + 1) * 512],
                            start=(kjp == 0), stop=(kjp == 6), perf_mode=DR)
                    t = _t(hu_p, [HD + 1, 512], BF16, f"hu{hp}{hh}{ni}")
                    nc.vector.tensor_copy(out=t, in_=pav[:])
                    # sums row scattered into [128, 4] column block r
                    r = 4 * hp + 2 * hh + ni
                    dq.dma_start(out=stk[:, 4 * r:4 * r + 4], in_=t[HD:HD + 1, :])
                    hu_all[(hp, hh, ni)] = t

        def emit_norm_batch(b, hu_all, stk, hsb, dq):
            """reciprocal of sums in [128, 64] form + dram-bounce broadcast."""
            skf = _t(sk_p, [128, 64], F32, "skf", bufs=2)
            nc.vector.tensor_copy(out=skf, in_=stk[:])
            rf = _t(sk_p, [128, 64], F32, "rf", bufs=2)
            nc.vector.reciprocal(out=rf[:], in_=skf[:])
            rbf = _t(sk_p, [128, 64], BF16, "rbf", bufs=2)
            nc.vector.tensor_copy(out=rbf, in_=rf[:])
            rd = _t(dram_p, [128, 64], BF16, "rd", bufs=2)
            dq.dma_start(out=rd[:, :], in_=rbf[:])
            for hp in range(4):
                for hh in range(2):
                    for ni in range(2):
                        bc = _t(sk_p, [HD, 512], BF16, "bc", bufs=2)
                        r = 4 * hp + 2 * hh + ni
                        dq.dma_start(out=bc,
                                     in_=_ap(rd, 4 * r,
                                             [[0, HD], [64, 128], [1, 4]]))
                        nc.vector.tensor_tensor(
                            out=hsb[64 * hh:64 * hh + 64, hp,
                                    ni * 512:(ni + 1) * 512],
                            in0=hu_all[(hp, hh, ni)][0:HD, :], in1=bc[:],
                            op=ALU.mult)

        def emit_proj(b, hsb):
            for nj in range(2):
                for oi in range(4):
                    pp = _t(pa_ps, [128, 512], F32, "pa")
                    for kip in range(0, 4, 2):
                        nc.tensor.matmul(
                            pp[:], wTp[:, kip:kip + 2, oi * 128:(oi + 1) * 128],
                            hsb[:, kip:kip + 2, nj * 512:(nj + 1) * 512],
                            start=(kip == 0), stop=(kip == 2), perf_mode=DR)
                    xr = _t(xr_p, [128, 512], F32, "xr")
                    nc.sync.dma_start(
                        out=xr, in_=x_d[b, oi * 128:(oi + 1) * 128,
                                        nj * 512:(nj + 1) * 512])
                    nc.vector.scalar_tensor_tensor(
                        out=xr, in0=pp[:], scalar=bpe_sb[:, oi:oi + 1], in1=xr[:],
                        op0=ALU.add, op1=ALU.add)
                    nc.scalar.dma_start(
                        out=out_d[b, oi * 128:(oi + 1) * 128,
                                  nj * 512:(nj + 1) * 512],
                        in_=xr[:])

        # ================= schedule =================
        # DMA queues: gpsimd / sync / scalar / vector / tensor
        xt0 = emit_xload(0, [nc.gpsimd, nc.sync, nc.gpsimd, nc.sync])
        W_ORDER = [0, 4, 1, 5, 2, 6, 3, 7, 8, 9, 10, 11, 12, 13, 14, 15]
        wst = {}
        for oi in [0, 4, 1, 5]:
            wst[oi] = emit_wslab_dma(oi, nc.sync)
        s01_0 = emit_stats(0, xt0)
        for oi in [0, 4, 1, 5]:
            emit_wslab_transpose(oi, wst[oi])
        xt1 = emit_xload(1, [nc.gpsimd, nc.sync, nc.gpsimd, nc.sync])
        for oi in [2, 6, 3, 7]:
            wst[oi] = emit_wslab_dma(oi, nc.sync)
            emit_wslab_transpose(oi, wst[oi])
        nt0 = [emit_norm_i(0, xt0, s01_0, i) for i in range(4)]
        s01_1 = emit_stats(1, xt1)
        for oi in [8, 9, 10, 11]:
            wst[oi] = emit_wslab_dma(oi, nc.sync)
            emit_wslab_transpose(oi, wst[oi])
        for oi in [12, 13, 14, 15]:
            wst[oi] = emit_wslab_dma(oi, nc.scalar)
            emit_wslab_transpose(oi, wst[oi])

        qk0 = {}
        hsb0 = _t(h_p, [128, 4, 1024], FP8, "hsb")
        hsb1 = _t(h_p, [128, 4, 1024], FP8, "hsb")
        DQ = nc.gpsimd
        hu0, hu1 = {}, {}
        stk0 = _t(sk_p, [128, 64], BF16, "stk", bufs=2)
        vx0 = emit_vinit(0)

        emit_qk_oi(nt0, qk0, 0); emit_qk_oi(nt0, qk0, 4)
        p00 = emit_spair(qk0, 0, DVE_EXP)
        emit_qk_oi(nt0, qk0, 1); emit_qk_oi(nt0, qk0, 5)
        for ni in range(4): emit_v_ni(nt0, vx0, ni)
        p01 = emit_spair(qk0, 1, DVE_EXP)
        for ni in range(4, 8): emit_v_ni(nt0, vx0, ni)
        emit_qk_oi(nt0, qk0, 2); emit_qk_oi(nt0, qk0, 6)
        emit_avpair(vx0, p00, 0, stk0, hu0, DQ)
        p02 = emit_spair(qk0, 2, DVE_EXP)
        emit_qk_oi(nt0, qk0, 3); emit_qk_oi(nt0, qk0, 7)
        emit_avpair(vx0, p01, 1, stk0, hu0, DQ)
        p03 = emit_spair(qk0, 3, DVE_EXP)
        nt1 = [emit_norm_i(1, xt1, s01_1, i) for i in range(4)]
        emit_avpair(vx0, p02, 2, stk0, hu0, DQ)
        qk1 = {}
        vx1 = emit_vinit(1)
        emit_qk_oi(nt1, qk1, 0); emit_qk_oi(nt1, qk1, 4)
        p10 = emit_spair(qk1, 0, DVE_EXP)
        emit_avpair(vx0, p03, 3, stk0, hu0, DQ)
        emit_qk_oi(nt1, qk1, 1); emit_qk_oi(nt1, qk1, 5)
        emit_norm_batch(0, hu0, stk0, hsb0, DQ)
        p11 = emit_spair(qk1, 1, DVE_EXP)
        for ni in range(8): emit_v_ni(nt1, vx1, ni)
        stk1 = _t(sk_p, [128, 64], BF16, "stk", bufs=2)
        emit_qk_oi(nt1, qk1, 2); emit_qk_oi(nt1, qk1, 6)
        emit_proj(0, hsb0)
        emit_avpair(vx1, p10, 0, stk1, hu1, DQ)
        emit_qk_oi(nt1, qk1, 3); emit_qk_oi(nt1, qk1, 7)
        p12 = emit_spair(qk1, 2, DVE_EXP_P2)
        emit_avpair(vx1, p11, 1, stk1, hu1, DQ)
        p13 = emit_spair(qk1, 3, DVE_EXP_P3)
        emit_avpair(vx1, p12, 2, stk1, hu1, DQ)
        emit_avpair(vx1, p13, 3, stk1, hu1, DQ)
        emit_norm_batch(1, hu1, stk1, hsb1, DQ)
        emit_proj(1, hsb1)


def _split_multi_waits(nc, limit=1):
    """This walrus build rejects >1 sync wait per instruction; hoist extras
    onto same-engine NoOps inserted immediately before."""
    n = 0
    for f in nc.m.functions:
        for bb in f.blocks:
            insts = list(bb.instructions)
            changed = False
            new = []
            for inst in insts:
                si = inst.sync_info
                waits = list(si.on_wait) if si is not None else []
                if len(waits) > limit:
                    extra, keep = waits[:-limit], waits[-limit:]
                    for w in extra:
                        nop = mybir.InstNoOp(
                            name=f"wsplit-{n}", engine=inst.engine, ins=[], outs=[],
                            sync_info=mybir.SyncInfo(on_wait=[w], on_update=[]))
                        new.append(nop)
                        n += 1
                    inst.sync_info = mybir.SyncInfo(
                        on_wait=keep, on_update=list(si.on_update))
                    changed = True
                new.append(inst)
            if changed:
                bb.instructions = new


_NC_CACHE = None


def _get_nc():
    global _NC_CACHE
    if _NC_CACHE is None:
        _NC_CACHE = build_bass()
    return _NC_CACHE


def _run(inputs, **kw):
    x = np.ascontiguousarray(np.asarray(inputs["x"], dtype=np.float32))
    norm_scale = np.asarray(inputs["norm_scale"], dtype=np.float32)
    norm_bias = np.asarray(inputs["norm_bias"], dtype=np.float32)
    w_qkv = np.ascontiguousarray(np.asarray(inputs["w_qkv"], dtype=np.float32))
    b_qkv = np.asarray(inputs["b_qkv"], dtype=np.float32)
    w_proj = np.ascontiguousarray(np.asarray(inputs["w_proj"], dtype=np.float32))
    b_proj = np.asarray(inputs["b_proj"], dtype=np.float32)

    Bf, Cf, Hf, Wf = x.shape
    xf = x.reshape(Bf, Cf, Hf * Wf)
    import ml_dtypes
    xfb = xf.astype(ml_dtypes.bfloat16)
    bpe = (b_proj + w_proj @ b_qkv[2 * C:3 * C]).astype(np.float32)
    bqk = np.ascontiguousarray(b_qkv[0:2 * C])

    nc = _get_nc()
    in_maps = []
    for c in range(NCORES):
        in_maps.append({
            "xb": np.ascontiguousarray(xfb[c * B_LOC:(c + 1) * B_LOC]),
            "x": np.ascontiguousarray(xf[c * B_LOC:(c + 1) * B_LOC]),
            "wqkv": w_qkv,
            "bqk": bqk,
            "gam": np.ascontiguousarray(norm_scale),
            "bet": np.ascontiguousarray(norm_bias),
            "wproj": w_proj,
            "bpe": bpe,
        })
    res = run_bass_kernel_spmd(nc, in_maps, core_ids=list(range(NCORES)), **kw)
    out = np.concatenate([res.results[c]["out"] for c in range(NCORES)], axis=0)
    return out.reshape(Bf, Cf, Hf, Wf), res


def kernel(**inputs) -> np.ndarray:
    out, _ = _run(inputs)
    return out


# revision 30
# speedup vs baseline: 1.8666x; 1.8666x over previous
"""AttentionBlock (GroupNorm + 8-head self-attention + proj + residual) on 8 TRN2 cores.

Sharding: pure data-parallel over batch. B=16 -> 2 images per core, no collectives.

Per-core pipeline (matmuls bf16, fp32 PSUM):
  - Weights transposed on the TENSOR engine (fp32 PE transpose via identity into
    PSUM, ACT evacuates+casts to one bf16 [128, ci, o] SBUF tile) -- no slow
    DMA-xbar transposes, and the transposes warm up the PE clock.
  - x streamed in bf16 (host-cast side input) for groupnorm stats + normalize;
    f32 x re-read only for the residual add.
  - GroupNorm stats: free-axis reduce + square-accumulate fused on DVE
    (tensor_tensor_reduce), 16-channel group sums via XOR stream_shuffle
    butterfly, rsqrt via bit-hack + Newton (all DVE, ACT keeps only exp).
  - QKV: q,k as [o, n]; v pre-transposed as [n, o_v] with a ones column for
    softmax row sums. PSUM evacuation split across DVE and ACT.
  - S^T = k^T q per head pair with tile_position row packing (concurrent 64-row
    streams); exp straight out of PSUM: most tiles on ACT (Exp activation),
    a few per pair on DVE via a Schraudolph int16 bit-trick (|S|/8 < ~80 safe).
  - A@V accumulates h^T (+sums row 64); spilled bf16 to SBUF promptly.
    Row-sum rows are gathered by SBUF->SBUF DMA into per-pair stacks,
    reciprocal via DVE reciprocal_approx_fast, broadcast along partitions by
    stride-0 SBUF DMA, one DVE multiply normalizes.
  - proj + residual (+bpe = b_proj + w_proj @ b_v folded on host).
"""

import numpy as np

import concourse.bass as bass
import concourse.tile as tile
from concourse import mybir, masks
from concourse.bass_utils import run_bass_kernel_spmd

F32 = mybir.dt.float32
I8 = mybir.dt.int8
I16 = mybir.dt.int16
I32 = mybir.dt.int32
BF16 = mybir.dt.bfloat16
FP8 = mybir.dt.float8e4
AX = mybir.AxisListType
ALU = mybir.AluOpType
ACTF = mybir.ActivationFunctionType
DR = mybir.MatmulPerfMode.DoubleRow

B_LOC = 2      # batch elements per core
C = 512
N = 1024       # H*W
NH = 8
HD = 64
G = 32         # groups
EPS = 1e-5
NCORES = 8

# P~ = exp(S/8 - ESHIFT): keeps P~ <= ~e^3.5 < 448 (fp8e4m3 max; overflow
# would produce NaN since e4m3fn has no inf). The shift cancels in the
# softmax normalization.
ESHIFT = 3.0
# Schraudolph exp -> fp8e4m3 bits: round((s/8 - ESHIFT)*(8/ln2) + (56 - 0.46)),
# written via uint8 so negative args saturate to 0 (= fp8 +0).
SCH_MUL = float(8.0 / np.log(2.0) / 8.0)
SCH_BIAS = float(7.0 * 8.0 - 0.46 - ESHIFT * 8.0 / np.log(2.0))
# (hh, mj) S^T tiles computed on DVE instead of ACT, per pair (load balance);
# heavier DVE share on the last pairs of batch 1 to shorten the exp tail.
DVE_EXP = {(1, 7)}
DVE_EXP_P2 = {(0, 6), (1, 6), (0, 7), (1, 7)}
DVE_EXP_P3 = {(0, 4), (1, 4), (0, 5), (1, 5), (0, 6), (1, 6), (0, 7), (1, 7)}


def _ap(t, offset_elems, pattern):
    a = t[:] if hasattr(t, "shape") else t
    return bass.AP(tensor=a.tensor, offset=a.offset + offset_elems, ap=pattern)


def _t(pool, shape, dt, tag, bufs=None):
    return pool.tile(shape, dt, tag=tag, name=tag, bufs=bufs)


def build_bass(split=True):
    nc = bass.Bass()
    xb_d = nc.declare_dram_parameter("xb", [B_LOC, C, N], BF16, isOutput=False)
    x_d = nc.declare_dram_parameter("x", [B_LOC, C, N], F32, isOutput=False)
    wq_d = nc.declare_dram_parameter("wqkv", [3 * C, C], F32, isOutput=False)
    bqk_d = nc.declare_dram_parameter("bqk", [2 * C], F32, isOutput=False)
    gam_d = nc.declare_dram_parameter("gam", [C], F32, isOutput=False)
    bet_d = nc.declare_dram_parameter("bet", [C], F32, isOutput=False)
    wp_d = nc.declare_dram_parameter("wproj", [C, C], F32, isOutput=False)
    bpe_d = nc.declare_dram_parameter("bpe", [C], F32, isOutput=False)
    out_d = nc.declare_dram_parameter("out", [B_LOC, C, N], F32, isOutput=True)

    with tile.TileContext(nc) as tc:
        _build_tile(tc, xb_d, x_d, wq_d, bqk_d, gam_d, bet_d, wp_d, bpe_d, out_d)
    if split:
        _split_multi_waits(nc)
    return nc


def _build_tile(tc, xb_d, x_d, wq_d, bqk_d, gam_d, bet_d, wp_d, bpe_d, out_d):
    nc = tc.nc
    from contextlib import ExitStack
    ctx = ExitStack()
    with ctx:
        wpool = ctx.enter_context(tc.tile_pool(name="wpool", bufs=1))
        xt_p = ctx.enter_context(tc.tile_pool(name="xt", bufs=2))
        nt_p = ctx.enter_context(tc.tile_pool(name="nt", bufs=2))
        qk_p = ctx.enter_context(tc.tile_pool(name="qk", bufs=1))
        v_p = ctx.enter_context(tc.tile_pool(name="vx", bufs=2))
        pt_p = ctx.enter_context(tc.tile_pool(name="pt", bufs=3))
        h_p = ctx.enter_context(tc.tile_pool(name="hs", bufs=2))
        hu_p = ctx.enter_context(tc.tile_pool(name="hu", bufs=1))
        st_p = ctx.enter_context(tc.tile_pool(name="stats", bufs=4))
        sk_p = ctx.enter_context(tc.tile_pool(name="sk", bufs=2))
        xr_p = ctx.enter_context(tc.tile_pool(name="xr", bufs=2))
        wst_p = ctx.enter_context(tc.tile_pool(name="wstage", bufs=2))
        dram_p = ctx.enter_context(tc.tile_pool(name="drp", bufs=1, space="DRAM"))
        pa_ps = ctx.enter_context(tc.tile_pool(name="pa", bufs=2, space="PSUM"))
        st_ps = ctx.enter_context(tc.tile_pool(name="stp", bufs=2, space="PSUM"))
        av_ps = ctx.enter_context(tc.tile_pool(name="avp", bufs=2, space="PSUM"))

        # ---- constants ----
        bqk_sb = _t(wpool, [128, 8], F32, "bqk")
        nc.gpsimd.dma_start(out=bqk_sb, in_=_ap(bqk_d, 0, [[1, 128], [128, 8]]))
        gam_sb = _t(wpool, [128, 4], F32, "gam")
        nc.gpsimd.dma_start(out=gam_sb, in_=_ap(gam_d, 0, [[1, 128], [128, 4]]))
        bet_sb = _t(wpool, [128, 4], F32, "bet")
        nc.gpsimd.dma_start(out=bet_sb, in_=_ap(bet_d, 0, [[1, 128], [128, 4]]))
        bpe_sb = _t(wpool, [128, 4], F32, "bpe")
        nc.gpsimd.dma_start(out=bpe_sb, in_=_ap(bpe_d, 0, [[1, 128], [128, 4]]))
        # identity via DMA diagonal scatter (affine_select is unsupported here)
        eshift_sb = _t(wpool, [128, 1], F32, "eshift")
        nc.vector.memset(eshift_sb[:], -ESHIFT)
        ident = _t(wpool, [128, 128], F32, "ident")
        id_dram = _t(dram_p, [128, 128], F32, "idd", bufs=1)
        nc.vector.memset(ident[:], 0.0)
        ones_row = _t(wpool, [1, 128], F32, "ones_row")
        nc.vector.memset(ones_row[:], 1.0)
        nc.gpsimd.dma_start(out=id_dram[:, :], in_=ident[:])
        nc.gpsimd.dma_start(out=_ap(id_dram, 0, [[129, 128]]), in_=ones_row[:])
        nc.gpsimd.dma_start(out=ident[:], in_=id_dram[:, :])

        # ---- x loads (bf16), spread across queues ----
        def emit_xload(b, engs):
            xt = []
            for i in range(4):
                t = _t(xt_p, [128, 1024], BF16, f"x{i}")
                engs[i % len(engs)].dma_start(out=t, in_=xb_d[b, i * 128:(i + 1) * 128, :])
                xt.append(t)
            return xt

        # ---- weights: DMA f32 slab -> 4 PE transposes -> ACT evac+cast ----
        # wT[c_in_chunk, ci, o] with o: 0:512 q | 512:1024 k | 1024:1536 v (bf16)
        # wTp[c_in_chunk, ci, o_proj] fp8 for the DoubleRow proj matmuls
        wT = _t(wpool, [128, 4, 1536], BF16, "wT")
        wTp = _t(wpool, [128, 4, 512], FP8, "wTp")

        def emit_wslab_dma(oi, eng):
            t = _t(wst_p, [128, 512], F32, "wst")
            if oi < 12:
                eng.dma_start(out=t, in_=wq_d[oi * 128:(oi + 1) * 128, :])
            else:
                eng.dma_start(out=t, in_=wp_d[(oi - 12) * 128:(oi - 11) * 128, :])
            return t

        def emit_wslab_transpose(oi, t):
            ps = _t(pa_ps, [128, 512], F32, "pa")
            for ci in range(4):
                nc.tensor.transpose(ps[:, ci * 128:(ci + 1) * 128],
                                    t[:, ci * 128:(ci + 1) * 128], ident[:])
            if oi < 12:
                dst = wT[:, :, oi * 128:(oi + 1) * 128]
            else:
                dst = wTp[:, :, (oi - 12) * 128:(oi - 11) * 128]
            nc.scalar.copy(out=dst, in_=ps[:].rearrange("p (c o) -> p c o", c=4))

        # ---- GroupNorm stats -> per-channel (s0, s1) affine scalars ----
        # All 4 channel-chunks batched: one [128, 2, 4] sums tile, one
        # butterfly, one [128, 4] rsqrt Newton chain.
        def emit_stats(b, xt):
            c = _t(st_p, [128, 2, 4], F32, f"cs{b}")
            for i in range(4):
                nc.vector.reduce_sum(out=c[:, 0, i:i + 1], in_=xt[i][:], axis=AX.X)
                scr = _t(st_p, [128, 1024], BF16, "sqscr", bufs=1)
                nc.scalar.activation(out=scr[:], in_=xt[i][:], func=ACTF.Square,
                                     accum_out=c[:, 1, i:i + 1])
            sh = _t(st_p, [128, 2, 4], F32, f"sh{b}")
            for s in (8, 4, 2, 1):
                nc.vector.stream_shuffle(out=sh, in_=c[:],
                                         mask=[j ^ s for j in range(32)])
                nc.vector.tensor_add(out=c, in0=c[:], in1=sh[:])
            mmt = _t(st_p, [128, 2, 4], F32, f"mmt{b}")
            nc.vector.tensor_scalar_mul(out=mmt, in0=c[:], scalar1=1.0 / (16 * N))
            u = _t(st_p, [128, 4], F32, f"u{b}")
            nc.vector.tensor_mul(out=u, in0=mmt[:, 0, :], in1=mmt[:, 0, :])
            nc.vector.tensor_sub(out=u, in0=mmt[:, 1, :], in1=u[:])
            nc.vector.tensor_scalar_add(out=u, in0=u[:], scalar1=EPS)
            ri = _t(st_p, [128, 4], I32, f"ri{b}")
            nc.vector.tensor_scalar(out=ri, in0=u[:].bitcast(I32), scalar1=1,
                                    scalar2=None, op0=ALU.logical_shift_right)
            nc.vector.tensor_scalar(out=ri, in0=ri[:], scalar1=-1,
                                    scalar2=0x5F3759DF, op0=ALU.mult, op1=ALU.add)
            r = ri[:].bitcast(F32)
            h = _t(st_p, [128, 4], F32, f"h{b}")
            nc.vector.tensor_scalar_mul(out=h, in0=u[:], scalar1=0.5)
            t2 = _t(st_p, [128, 4], F32, f"t2{b}")
            for _ in range(2):
                nc.vector.tensor_mul(out=t2, in0=r, in1=r)
                nc.vector.tensor_mul(out=t2, in0=h[:], in1=t2[:])
                nc.vector.tensor_scalar(out=t2, in0=t2[:], scalar1=-1.0,
                                        scalar2=1.5, op0=ALU.mult, op1=ALU.add)
                nc.vector.tensor_mul(out=r, in0=r, in1=t2[:])
            s0 = _t(st_p, [128, 4], F32, f"s0{b}")
            nc.vector.tensor_mul(out=s0, in0=gam_sb[:], in1=r)
            t1 = _t(st_p, [128, 4], F32, f"t1{b}")
            nc.vector.tensor_mul(out=t1, in0=mmt[:, 0, :], in1=s0[:])
            s1 = _t(st_p, [128, 4], F32, f"s1{b}")
            nc.vector.tensor_sub(out=s1, in0=bet_sb[:], in1=t1[:])
            return [(s0[:, i:i + 1], s1[:, i:i + 1]) for i in range(4)]

        def emit_norm_i(b, xt, s01, i):
            t = _t(nt_p, [128, 1024], BF16, f"n{i}")
            nc.vector.tensor_scalar(
                out=t[:], in0=xt[i][:], scalar1=s01[i][0], scalar2=s01[i][1],
                op0=ALU.mult, op1=ALU.add)
            return t

        def emit_qk_oi(nt, qk, oi, evac_scalar=False):
            t = _t(qk_p, [128, 1024], BF16, f"qk{oi}")
            for nj in range(2):
                pk = _t(pa_ps, [128, 512], F32, "pa")
                for ki in range(4):
                    nc.tensor.matmul(
                        pk[:], wT[:, ki, oi * 128:(oi + 1) * 128],
                        nt[ki][:, nj * 512:(nj + 1) * 512],
                        start=(ki == 0), stop=(ki == 3))
                if evac_scalar:
                    nc.scalar.add(out=t[:, nj * 512:(nj + 1) * 512], in_=pk[:],
                                  add=bqk_sb[:, oi:oi + 1])
                else:
                    nc.vector.tensor_scalar_add(
                        out=t[:, nj * 512:(nj + 1) * 512], in0=pk[:],
                        scalar1=bqk_sb[:, oi:oi + 1])
            qk[oi] = t

        def emit_vinit(b):
            """vx layout [128(kpos), kj 8, head 8, 80] fp8; col 64 = ones for
            softmax row sums, cols 65:80 pad for DoubleRow 16B stride."""
            t = _t(v_p, [128, 8, NH, 80], FP8, "vxb")
            nc.vector.memset(t[:, :, :, HD:HD + 1], 1.0)
            return t

        def emit_v_ni(nt, vxb, ni):
            pv = _t(pa_ps, [128, 512], F32, "pa")
            for ki in range(4):
                nc.tensor.matmul(
                    pv[:], nt[ki][:, ni * 128:(ni + 1) * 128],
                    wT[:, ki, 1024:1536],
                    start=(ki == 0), stop=(ki == 3))
            nc.vector.tensor_copy(
                out=vxb[:, ni, :, 0:HD],
                in_=pv[:].rearrange("p (h d) -> p h d", h=NH))

        def emit_spair(qk, hp, dve_set):
            """S^T + exp for head pair hp -> P~ fp8 tiles [128, mj 8, 1024]."""
            pb = [_t(pt_p, [128, 8, 1024], FP8, f"ptb{hh}") for hh in range(2)]
            for mj in range(8):
                stt = [None, None]
                for hh in range(2):
                    base = 64 * hh
                    stt[hh] = _t(st_ps, [128, 1024], F32, "st")
                    for ni in range(2):
                        nc.tensor.matmul(
                            stt[hh][:, ni * 512:(ni + 1) * 512],
                            qk[4 + hp][base:base + 64, mj * 128:(mj + 1) * 128],
                            qk[hp][base:base + 64, ni * 512:(ni + 1) * 512],
                            start=True, stop=True, tile_position=(base, 0))
                for hh in range(2):
                    p = pb[hh][:, mj, :]
                    if (hh, mj) in dve_set:
                        nc.vector.tensor_scalar(
                            out=p.bitcast(mybir.dt.uint8), in0=stt[hh][:],
                            scalar1=SCH_MUL, scalar2=SCH_BIAS,
                            op0=ALU.mult, op1=ALU.add)
                    else:
                        nc.scalar.activation(out=p, in_=stt[hh][:], func=ACTF.Exp,
                                             scale=float(HD) ** -0.5,
                                             bias=eshift_sb[:])
            return pb

        def emit_avpair(vxb, pb, hp, stk, hu_all, dq):
            """A@V (fp8) -> spill h^T (unnormalized) + sums row to stack."""
            for ni in range(2):
                for hh in range(2):
                    pav = _t(av_ps, [HD + 1, 512], F32, "av")
                    for kj in range(8):
                        nc.tensor.matmul(
                            pav[:], vxb[:, kj, 2 * hp + hh, 0:HD + 1],
                            pb[hh][:, kj, ni * 512:(ni + 1) * 512],
                            start=(kj == 0), stop=(kj == 7))
                    t = _t(hu_p, [HD + 1, 512], BF16, f"hu{hp}{hh}{ni}")
                    nc.vector.tensor_copy(out=t, in_=pav[:])
                    r = 4 * hp + 2 * hh + ni
                    dq.dma_start(out=stk[r:r + 1, :], in_=t[HD:HD + 1, :])
                    hu_all[(hp, hh, ni)] = t

        def emit_norm_batch(b, hu_all, stk, hsb, dq):
            """reciprocal of all 16 sum rows + dram-bounce broadcast."""
            skf = _t(sk_p, [16, 512], F32, "skf", bufs=2)
            nc.vector.tensor_copy(out=skf, in_=stk[:])
            rf = _t(sk_p, [16, 512], F32, "rf", bufs=2)
            nc.vector.reciprocal(out=rf[:], in_=skf[:])
            rbf = _t(sk_p, [16, 512], BF16, "rbf", bufs=2)
            nc.vector.tensor_copy(out=rbf, in_=rf[:])
            rd = _t(dram_p, [16, 512], BF16, "rd", bufs=2)
            nc.sync.dma_start(out=rd[:, :], in_=rbf[:])
            for hp in range(4):
                for hh in range(2):
                    for ni in range(2):
                        bc = _t(sk_p, [HD, 512], BF16, "bc", bufs=2)
                        r = 4 * hp + 2 * hh + ni
                        nc.sync.dma_start(
                            out=bc, in_=_ap(rd, r * 512, [[0, HD], [1, 512]]))
                        nc.vector.tensor_tensor(
                            out=hsb[64 * hh:64 * hh + 64, hp,
                                    ni * 512:(ni + 1) * 512],
                            in0=hu_all[(hp, hh, ni)][0:HD, :], in1=bc[:],
                            op=ALU.mult)

        def emit_proj(b, hsb):
            for nj in range(2):
                for oi in range(4):
                    pp = _t(pa_ps, [128, 512], F32, "pa")
                    for ki in range(4):
                        nc.tensor.matmul(
                            pp[:], wTp[:, ki, oi * 128:(oi + 1) * 128],
                            hsb[:, ki, nj * 512:(nj + 1) * 512],
                            start=(ki == 0), stop=(ki == 3))
                    xr = _t(xr_p, [128, 512], F32, "xr")
                    nc.sync.dma_start(
                        out=xr, in_=x_d[b, oi * 128:(oi + 1) * 128,
                                        nj * 512:(nj + 1) * 512])
                    nc.vector.scalar_tensor_tensor(
                        out=xr, in0=pp[:], scalar=bpe_sb[:, oi:oi + 1], in1=xr[:],
                        op0=ALU.add, op1=ALU.add)
                    nc.sync.dma_start(
                        out=out_d[b, oi * 128:(oi + 1) * 128,
                                  nj * 512:(nj + 1) * 512],
                        in_=xr[:])

        # ================= schedule =================
        # DMA queues: gpsimd / sync / scalar / vector / tensor
        xt0 = emit_xload(0, [nc.gpsimd, nc.sync, nc.gpsimd, nc.sync])
        W_ORDER = [0, 4, 1, 5, 2, 6, 3, 7, 8, 9, 10, 11, 12, 13, 14, 15]
        wst = {}
        for oi in [0, 4, 1, 5]:
            wst[oi] = emit_wslab_dma(oi, nc.sync)
        s01_0 = emit_stats(0, xt0)
        for oi in [0, 4, 1, 5]:
            emit_wslab_transpose(oi, wst[oi])
        xt1 = emit_xload(1, [nc.gpsimd, nc.sync, nc.gpsimd, nc.sync])
        for oi in [2, 6, 3, 7]:
            wst[oi] = emit_wslab_dma(oi, nc.sync)
            emit_wslab_transpose(oi, wst[oi])
        nt0 = [emit_norm_i(0, xt0, s01_0, i) for i in range(4)]
        s01_1 = emit_stats(1, xt1)
        for oi in [8, 9, 10, 11]:
            wst[oi] = emit_wslab_dma(oi, nc.sync)
            emit_wslab_transpose(oi, wst[oi])
        for oi in [12, 13, 14, 15]:
            wst[oi] = emit_wslab_dma(oi, nc.scalar)
            emit_wslab_transpose(oi, wst[oi])

        qk0 = {}
        hsb0 = _t(h_p, [128, 4, 1024], FP8, "hsb")
        hsb1 = _t(h_p, [128, 4, 1024], FP8, "hsb")
        DQ = nc.gpsimd
        hu0, hu1 = {}, {}
        stk0 = _t(sk_p, [16, 512], BF16, "stk", bufs=2)
        vx0 = emit_vinit(0)

        emit_qk_oi(nt0, qk0, 0); emit_qk_oi(nt0, qk0, 4)
        p00 = emit_spair(qk0, 0, DVE_EXP)
        emit_qk_oi(nt0, qk0, 1); emit_qk_oi(nt0, qk0, 5)
        for ni in range(4): emit_v_ni(nt0, vx0, ni)
        p01 = emit_spair(qk0, 1, DVE_EXP)
        for ni in range(4, 8): emit_v_ni(nt0, vx0, ni)
        emit_qk_oi(nt0, qk0, 2); emit_qk_oi(nt0, qk0, 6)
        emit_avpair(vx0, p00, 0, stk0, hu0, DQ)
        p02 = emit_spair(qk0, 2, DVE_EXP)
        emit_qk_oi(nt0, qk0, 3); emit_qk_oi(nt0, qk0, 7)
        emit_avpair(vx0, p01, 1, stk0, hu0, DQ)
        p03 = emit_spair(qk0, 3, DVE_EXP)
        nt1 = [emit_norm_i(1, xt1, s01_1, i) for i in range(4)]
        emit_avpair(vx0, p02, 2, stk0, hu0, DQ)
        qk1 = {}
        vx1 = emit_vinit(1)
        emit_qk_oi(nt1, qk1, 0); emit_qk_oi(nt1, qk1, 4)
        p10 = emit_spair(qk1, 0, DVE_EXP)
        emit_avpair(vx0, p03, 3, stk0, hu0, DQ)
        emit_qk_oi(nt1, qk1, 1); emit_qk_oi(nt1, qk1, 5)
        emit_norm_batch(0, hu0, stk0, hsb0, DQ)
        p11 = emit_spair(qk1, 1, DVE_EXP)
        for ni in range(8): emit_v_ni(nt1, vx1, ni)
        stk1 = _t(sk_p, [16, 512], BF16, "stk", bufs=2)
        emit_qk_oi(nt1, qk1, 2); emit_qk_oi(nt1, qk1, 6)
        emit_proj(0, hsb0)
        emit_avpair(vx1, p10, 0, stk1, hu1, DQ)
        emit_qk_oi(nt1, qk1, 3); emit_qk_oi(nt1, qk1, 7)
        p12 = emit_spair(qk1, 2, DVE_EXP_P2)
        emit_avpair(vx1, p11, 1, stk1, hu1, DQ)
        p13 = emit_spair(qk1, 3, DVE_EXP_P3)
        emit_avpair(vx1, p12, 2, stk1, hu1, DQ)
        emit_avpair(vx1, p13, 3, stk1, hu1, DQ)
        emit_norm_batch(1, hu1, stk1, hsb1, DQ)
        emit_proj(1, hsb1)


def _split_multi_waits(nc, limit=1):
    """This walrus build rejects >1 sync wait per instruction; hoist extras
    onto same-engine NoOps inserted immediately before."""
    n = 0
    for f in nc.m.functions:
        for bb in f.blocks:
            insts = list(bb.instructions)
            changed = False
            new = []
            for inst in insts:
                si = inst.sync_info
                waits = list(si.on_wait) if si is not None else []
                if len(waits) > limit:
                    extra, keep = waits[:-limit], waits[-limit:]
                    for w in extra:
                        nop = mybir.InstNoOp(
                            name=f"wsplit-{n}", engine=inst.engine, ins=[], outs=[],
                            sync_info=mybir.SyncInfo(on_wait=[w], on_update=[]))
                        new.append(nop)
                        n += 1
                    inst.sync_info = mybir.SyncInfo(
                        on_wait=keep, on_update=list(si.on_update))
                    changed = True
                new.append(inst)
            if changed:
                bb.instructions = new


_NC_CACHE = None


def _get_nc():
    global _NC_CACHE
    if _NC_CACHE is None:
        _NC_CACHE = build_bass()
    return _NC_CACHE


def _run(inputs, **kw):
    x = np.ascontiguousarray(np.asarray(inputs["x"], dtype=np.float32))
    norm_scale = np.asarray(inputs["norm_scale"], dtype=np.float32)
    norm_bias = np.asarray(inputs["norm_bias"], dtype=np.float32)
    w_qkv = np.ascontiguousarray(np.asarray(inputs["w_qkv"], dtype=np.float32))
    b_qkv = np.asarray(inputs["b_qkv"], dtype=np.float32)
    w_proj = np.ascontiguousarray(np.asarray(inputs["w_proj"], dtype=np.float32))
    b_proj = np.asarray(inputs["b_proj"], dtype=np.float32)

    Bf, Cf, Hf, Wf = x.shape
    xf = x.reshape(Bf, Cf, Hf * Wf)
    import ml_dtypes
    xfb = xf.astype(ml_dtypes.bfloat16)
    bpe = (b_proj + w_proj @ b_qkv[2 * C:3 * C]).astype(np.float32)
    bqk = np.ascontiguousarray(b_qkv[0:2 * C])

    nc = _get_nc()
    in_maps = []
    for c in range(NCORES):
        in_maps.append({
            "xb": np.ascontiguousarray(xfb[c * B_LOC:(c + 1) * B_LOC]),
            "x": np.ascontiguousarray(xf[c * B_LOC:(c + 1) * B_LOC]),
            "wqkv": w_qkv,
            "bqk": bqk,
            "gam": np.ascontiguousarray(norm_scale),
            "bet": np.ascontiguousarray(norm_bias),
            "wproj": w_proj,
            "bpe": bpe,
        })
    res = run_bass_kernel_spmd(nc, in_maps, core_ids=list(range(NCORES)), **kw)
    out = np.concatenate([res.results[c]["out"] for c in range(NCORES)], axis=0)
    return out.reshape(Bf, Cf, Hf, Wf), res


def kernel(**inputs) -> np.ndarray:
    out, _ = _run(inputs)
    return out
